# revision 23
# baseline (speedup 1.0000x reference)
"""Distributed multi-head attention kernel for one TRN2 chip (8 NeuronCores).

Problem: x[2,2048,1024] -> qkv -> 16-head attention -> out proj, f32 I/O.

Sharding: 8 cores = 2 batches x 4 head-groups (4 heads each).
Core c: batch b=c//4, head group g=c%4 (heads 4g..4g+3).

Structure (evolved from a 330us baseline; ~291-295us measured):
 - Inputs host-packed so every DMA moves >=2KB/partition-line, sliced by
   need-order across both HWDGE queues + the SWDGE for late weights; xt
   n-slice 0 split per k-chunk over all three DMA queues so the first qk
   accumulation starts as soon as its first chunks land.
 - Fill work (remaining qk m/n units, v production, proj) emitted in one
   priority-demoted block, hand-interleaved in deadline order; attention
   s->exp->PV chain at normal priority.  PV matmuls are emitted PV_LAG
   chunk-pairs behind their exp so the in-order PE queue never parks on
   a PV whose pt isn't ready.
 - One combined AllGather per q-range (each AllGather rendezvous costs
   ~7-15us regardless of payload, so fewer is better); qt3 is gathered
   in halves so the tail pipeline stays short.
 - Each proj load is ONE strided DMA [128, 8, qw] instead of 8, so the
   gpsimd queue never head-of-line-blocks an AllGather trigger behind
   serialized load triggers (~630ns each).  Mid-kernel loads demoted;
   tail loads (ranges 3,4) at normal priority so they beat the demoted
   out-stores into the queue.
 - Out-stores inline per range on the gpsimd queue.

Device algorithm (per core), all matmuls bf16 with f32 PSUM accumulate:
 1) qkT = wqk.T @ xT   [512,2048]  (q weights/bias pre-scaled by
    1/sqrt(dh) on host; bias added during DVE eviction)
 2) v_aug = xT.T @ w_v [2048, 4*65] (bias via DVE; per-head ones column
    interleaved so the PV matmul also emits softmax denominators)
 3) per (pair, q-range): per chunk-pair: sT = k @ qT (row-tiled head
    pair), p = exp(sT) on ScalarE (PSUM->SBUF bf16), oT_aug += v_aug.T
    @ p (M=65: row 64 = denominator); normalize oT by 1/denom on DVE.
 4) per q-range: AllGather o per pair (128 rows); proj with w_proj
    chunks stationary: out.T[of, q] += wp.T @ o_full; + bias; DMA out
    [256, 2048] transposed (host un-transposes).
"""

import os
import sys
import types
import numpy as np
import ml_dtypes

import concourse.bass as bass
import concourse.mybir as mybir
import concourse.bacc as bacc
import concourse.tile as tile
from concourse.bass_utils import run_bass_kernel_spmd

BF16 = mybir.dt.bfloat16
F32 = mybir.dt.float32

B, N, D = 2, 2048, 1024
H, DH = 16, 64
SCALE = DH ** -0.5

P = 128                 # partitions
NT = 512                # token free-dim tile
KC = N // P             # 16 k-token chunks
QT = N // NT            # 4 q tiles
DC = D // P             # 8 d_model chunks
HPC = 4                 # heads per core
OF = HPC * DH           # 256 o-features per core
VW = HPC * (DH + 1)     # v_aug width (260): per head [v(64) | ones(1)]

CORE_IDS = list(range(8))
GROUPS = [[0, 1, 2, 3], [4, 5, 6, 7]]
LAST_RESULTS = None


def _install_ntff_shim():
    """Provide antenv.axon_hooks (absent from this image's antenv stub) so
    run_bass_kernel_spmd(trace=True) can reach the NTFF profiler in
    libaxon_pjrt.so. Only needed when profiling."""
    if "antenv.axon_hooks" in sys.modules:
        return
    try:
        from trn_agent_boot.trn_boot import _ntff_profile_via_ctypes
        hook = _ntff_profile_via_ctypes("/opt/axon/libaxon_pjrt.so")
    except Exception:
        hook = None
    mod = types.ModuleType("antenv.axon_hooks")
    mod._hook = hook
    mod.get_axon_ntff_profile_hook = lambda: mod._hook
    mod.set_axon_ntff_profile_hook = lambda h: setattr(mod, "_hook", h)
    sys.modules["antenv.axon_hooks"] = mod


def build_nc():
    nc = bacc.Bacc("TRN2", target_bir_lowering=False, debug=False, num_devices=8)

    # Host-packed inputs (k-chunk-major along free dim for wide DMA lines):
    #  xtp[n]  rows n*128..: [128, 8*512]  = xT[k*128:(k+1)*128, n*512:+512]
    #  wqkp[m] rows m*128..: [128, 8*128]  = wqk[k*128:(k+1)*128, m*128:+128]
    #  wvp: [128, 8*256], wpp: [128, 8*256]
    xtp_ext = nc.dram_tensor("xtp", [4 * P, DC * NT], BF16, kind="ExternalInput")
    wqkp_ext = nc.dram_tensor("wqkp", [4 * P, DC * P], BF16, kind="ExternalInput")
    wvp_ext = nc.dram_tensor("wvp", [P, DC * OF], BF16, kind="ExternalInput")
    wpp_ext = nc.dram_tensor("wpp", [P, DC * OF], BF16, kind="ExternalInput")
    bqk_ext = nc.dram_tensor("bqk", [P, 4], F32, kind="ExternalInput")
    bv_ext = nc.dram_tensor("bv", [1, OF], F32, kind="ExternalInput")
    bp_ext = nc.dram_tensor("bp", [P, 2], F32, kind="ExternalInput")
    # transposed output [of, q]; host transposes back
    out_ext = nc.dram_tensor("out", [OF, N], F32, kind="ExternalOutput")

    # AllGather buffers per q-range (qt 0,1,2 full; qt3 in halves — ranges
    # 3,4.  Pair-0's qt3 is computed as one full range feeding both half
    # buffers).  Combined per-range gathers: each AllGather rendezvous
    # costs ~7us regardless of payload, so fewer is better.
    QRANGES = [(0, NT), (NT, NT), (2 * NT, NT),
               (3 * NT, NT // 2), (3 * NT + NT // 2, NT // 2),
               (3 * NT, NT)]
    ag_in = [nc.dram_tensor(f"ag_in_{i}", [2 * P, qw], BF16)
             for i, (q0, qw) in enumerate(QRANGES[:5])]
    ag_out = [nc.dram_tensor(f"ag_out_{i}", [8 * P, qw], BF16)
              for i, (q0, qw) in enumerate(QRANGES[:5])]
    # dummy collective to absorb the bootstrap barrier at T=0
    agw_in = nc.dram_tensor("agw_in", [1, 16], BF16)
    agw_out = nc.dram_tensor("agw_out", [4, 16], BF16)

    with tile.TileContext(nc) as tc:
        with (
            tc.tile_pool(name="xt_pool", bufs=1) as xt_pool,
            tc.tile_pool(name="w_pool", bufs=1) as w_pool,
            tc.tile_pool(name="qk_pool", bufs=1) as qk_pool,
            tc.tile_pool(name="v_pool", bufs=1) as v_pool,
            tc.tile_pool(name="const_pool", bufs=1) as const_pool,
            tc.tile_pool(name="pt_pool", bufs=8) as pt_pool,
            tc.tile_pool(name="o_pool", bufs=4) as o_pool,
            tc.tile_pool(name="nrm_pool", bufs=2) as nrm_pool,
            tc.tile_pool(name="ofull_pool", bufs=4) as ofull_pool,
            tc.tile_pool(name="pr_pool", bufs=2) as pr_pool,
            tc.tile_pool(name="sw_pool", bufs=2, space="PSUM") as sw_pool,
            tc.tile_pool(name="po_pool", bufs=1, space="PSUM") as po_pool,
            tc.tile_pool(name="aux_pool", bufs=2, space="PSUM") as aux_pool,
        ):
            # ---- T=0 warmups ------------------------------------------------
            nc.gpsimd.collective_compute(
                "AllGather", mybir.AluOpType.bypass,
                replica_groups=GROUPS,
                ins=[agw_in.ap().opt()],
                outs=[agw_out.ap().opt()])
            warm_in = const_pool.tile([1, 16], F32)
            warm_out = const_pool.tile([1, 16], F32)
            nc.vector.memset(warm_in[:], 0.0)
            nc.scalar.activation(
                warm_out[:], warm_in[:], mybir.ActivationFunctionType.Exp)

            # ---- input loads, need-order.  sync/scalar HWDGE carry the
            # early-critical tensors; gpsimd SWDGE carries late weights.
            bqk_sb = const_pool.tile([P, 4], F32)
            nc.sync.dma_start(bqk_sb[:], bqk_ext[:])
            bv_row = const_pool.tile([1, OF], F32)
            nc.scalar.dma_start(bv_row[:], bv_ext[:])

            wqk_m = {}
            t = w_pool.tile([P, DC * P], BF16, name="wqk2")
            nc.sync.dma_start(t[:], wqkp_ext[2 * P:3 * P, :])
            wqk_m[2] = t
            t = w_pool.tile([P, DC * P], BF16, name="wqk0")
            nc.scalar.dma_start(t[:], wqkp_ext[0:P, :])
            wqk_m[0] = t
            xt_n = [xt_pool.tile([P, DC * NT], BF16, name=f"xt{n}")
                    for n in range(4)]
            # xt n0 split per k-chunk over all three DMA queues so
            # qk(2,0) kh0 starts as soon as its first chunks land
            for k in range(DC):
                eng = (nc.sync, nc.scalar, nc.gpsimd)[k % 3]
                eng.dma_start(xt_n[0][:, k * NT:(k + 1) * NT],
                              xtp_ext[0:P, k * NT:(k + 1) * NT])
            nc.sync.dma_start(xt_n[1][:], xtp_ext[P:2 * P, :])
            wv_sb = w_pool.tile([P, DC * OF], BF16, name="wv")
            nc.scalar.dma_start(wv_sb[:], wvp_ext[:])
            nc.sync.dma_start(xt_n[3][:], xtp_ext[3 * P:4 * P, :])
            nc.scalar.dma_start(xt_n[2][:], xtp_ext[2 * P:3 * P, :])
            for m in (3, 1):
                t = w_pool.tile([P, DC * P], BF16, name=f"wqk{m}")
                nc.gpsimd.dma_start(t[:], wqkp_ext[m * P:(m + 1) * P, :])
                wqk_m[m] = t
            wp_sb = w_pool.tile([P, DC * OF], BF16, name="wp")
            nc.gpsimd.dma_start(wp_sb[:], wpp_ext[:])
            bp_sb = const_pool.tile([P, 2], F32)
            nc.scalar.dma_start(bp_sb[:], bp_ext[:])
            bv_bc = const_pool.tile([P, OF], F32)
            nc.gpsimd.partition_broadcast(bv_bc[:], bv_row[:])

            def xt_sl(k, n):          # [128, 512] token slice n of d-chunk k
                return xt_n[n][:, k * NT:(k + 1) * NT]

            def xt_ksl(k, t):         # [128, 128] token chunk t of d-chunk k
                n, r = divmod(t, 4)
                return xt_n[n][:, k * NT + r * P:k * NT + (r + 1) * P]

            def wqk_sl(m, k):         # [128, 128] d-chunk k of m-tile m
                return wqk_m[m][:, k * P:(k + 1) * P]

            def wv_sl(k):             # [128, 256]
                return wv_sb[:, k * OF:(k + 1) * OF]

            def wp_sl(cg, h):         # [128, 128] wp rows cg*128.., cols h*128..
                return wp_sb[:, cg * OF + h * P:cg * OF + (h + 1) * P]

            # ---- phase B: qkT = wqk.T @ xT -> 4 tiles [128, 2048] bf16 ------
            # m=0: q heads 0-1, m=1: q heads 2-3, m=2: k heads 0-1, m=3: k 2-3
            qk_sb = [qk_pool.tile([P, N], BF16, name=f"qk{m}") for m in range(4)]

            def qk_units(m, n):
                """qkT (m, n-slice) as 2 units of 4 k-chunks; DVE eviction
                with bias on the last."""
                state = {}

                def make(kh):
                    def u():
                        if kh == 0:
                            state["ps"] = aux_pool.tile([P, NT], F32, name="aux")
                        for k in range(4 * kh, 4 * kh + 4):
                            nc.tensor.matmul(
                                state["ps"][:], wqk_sl(m, k), xt_sl(k, n),
                                start=(k == 0), stop=(k == DC - 1))
                        if kh == 1:
                            nc.vector.tensor_scalar_add(
                                qk_sb[m][:, n * NT:(n + 1) * NT],
                                state["ps"][:], bqk_sb[:, m:m + 1])
                    return u
                return [make(0), make(1)]

            # ---- phase C: v_aug [2048, 260] bf16 (ones interleaved) ---------
            v_sb = [v_pool.tile([P, VW], BF16, name=f"v{t}") for t in range(KC)]

            def v_units(t):
                state = {}

                def ua():
                    state["ps"] = aux_pool.tile([P, OF], F32, name="aux")
                    for k in range(4):
                        nc.tensor.matmul(
                            state["ps"][:], xt_ksl(k, t), wv_sl(k),
                            start=(k == 0), stop=False)

                def ub():
                    ps = state["ps"]
                    for k in range(4, 8):
                        nc.tensor.matmul(
                            ps[:], xt_ksl(k, t), wv_sl(k),
                            start=False, stop=(k == DC - 1))
                    vdst = v_sb[t][:, :].rearrange("p (h c) -> p h c", c=DH + 1)
                    nc.vector.tensor_add(
                        vdst[:, :, 0:DH],
                        ps[:, :].rearrange("p (h c) -> p h c", c=DH),
                        bv_bc[:, :].rearrange("p (h c) -> p h c", c=DH))
                    nc.vector.memset(vdst[:, :, DH:DH + 1], 1.0)
                return [ua, ub]

            # ---- phase D: attention per (pair, q-range) ---------------------
            # PV matmuls are emitted PV_LAG chunk-pairs behind their exp so
            # the in-order PE queue never parks on a PV whose pt isn't ready.
            PV_LAG = 3

            def attn_range(p, ri):
                q0, qw = QRANGES[ri]
                kt = qk_sb[2 + p]
                qt_ = qk_sb[p]
                hA, hB = 2 * p, 2 * p + 1
                qs = slice(q0, q0 + qw)
                po0 = po_pool.tile([DH + 1, qw], F32, name="po0")
                po1 = po_pool.tile([DH + 1, qw], F32, name="po1")
                pend = []

                def pv(c2, ptA, ptB):
                    def u():
                        c, c1 = 2 * c2, 2 * c2 + 1
                        st, sp = (c == 0), (c1 == KC - 1)
                        nc.tensor.matmul(
                            po0[:],
                            v_sb[c][:, hA * (DH + 1):(hA + 1) * (DH + 1)],
                            ptA[:, 0:qw], start=st, stop=False)
                        nc.tensor.matmul(
                            po0[:],
                            v_sb[c1][:, hA * (DH + 1):(hA + 1) * (DH + 1)],
                            ptA[:, qw:2 * qw], start=False, stop=sp)
                        nc.tensor.matmul(
                            po1[:],
                            v_sb[c][:, hB * (DH + 1):(hB + 1) * (DH + 1)],
                            ptB[:, 0:qw], start=st, stop=False)
                        nc.tensor.matmul(
                            po1[:],
                            v_sb[c1][:, hB * (DH + 1):(hB + 1) * (DH + 1)],
                            ptB[:, qw:2 * qw], start=False, stop=sp)
                    return u

                for c2 in range(KC // 2):
                    c, c1 = 2 * c2, 2 * c2 + 1
                    cs = slice(c * P, (c + 1) * P)
                    cs1 = slice(c1 * P, (c1 + 1) * P)
                    swA = sw_pool.tile([P, 2 * qw], F32, name="sw")
                    swB = sw_pool.tile([P, 2 * qw], F32, name="sw")
                    # interleave quadrants so adjacent MMs run concurrently
                    nc.tensor.matmul(swA[:, 0:qw], kt[0:64, cs],
                                     qt_[0:64, qs], tile_position=(0, 0),
                                     start=True, stop=True)
                    nc.tensor.matmul(swB[:, 0:qw], kt[64:128, cs],
                                     qt_[64:128, qs], tile_position=(64, 0),
                                     start=True, stop=True)
                    nc.tensor.matmul(swA[:, qw:2 * qw], kt[0:64, cs1],
                                     qt_[0:64, qs], tile_position=(0, 0),
                                     start=True, stop=True)
                    nc.tensor.matmul(swB[:, qw:2 * qw], kt[64:128, cs1],
                                     qt_[64:128, qs], tile_position=(64, 0),
                                     start=True, stop=True)
                    ptA = pt_pool.tile([P, 2 * qw], BF16, name="pt")
                    nc.scalar.activation(
                        ptA[:], swA[:], mybir.ActivationFunctionType.Exp)
                    ptB = pt_pool.tile([P, 2 * qw], BF16, name="pt")
                    nc.scalar.activation(
                        ptB[:], swB[:], mybir.ActivationFunctionType.Exp)
                    pend.append(pv(c2, ptA, ptB))
                    if len(pend) > PV_LAG:
                        pend.pop(0)()
                for u in pend:
                    u()
                # normalize by 1/denominator (psum row 64, per q token)
                ot = o_pool.tile([P, qw], BF16, name="o")
                for hi, po in ((0, po0), (1, po1)):
                    d = nrm_pool.tile([1, qw], F32, name="d")
                    nc.vector.tensor_copy(d[0:1, :], po[64:65, :])
                    r = nrm_pool.tile([1, qw], F32, name="r")
                    scr = nrm_pool.tile([1, qw], F32, name="scr")
                    nc.vector.reciprocal_approx_accurate(
                        r[0:1, :], d[0:1, :], scr[0:1, :])
                    rb = nrm_pool.tile([64, qw], F32, name="rb")
                    nc.gpsimd.partition_broadcast(rb[0:64, :], r[0:1, :])
                    nc.vector.tensor_mul(
                        ot[64 * hi:64 * (hi + 1), :], po[0:64, :], rb[0:64, :])
                if ri == 5:
                    # pair-0 qt3 (full width) feeds both half-range buffers
                    nc.sync.dma_start(ag_in[3][0:P, :], ot[:, 0:NT // 2])
                    nc.sync.dma_start(ag_in[4][0:P, :], ot[:, NT // 2:NT])
                else:
                    nc.sync.dma_start(ag_in[ri][p * P:(p + 1) * P, :], ot[:])

            def ag_range(ri):
                nc.gpsimd.collective_compute(
                    "AllGather", mybir.AluOpType.bypass,
                    replica_groups=GROUPS,
                    ins=[ag_in[ri].ap().opt()],
                    outs=[ag_out[ri].ap().opt()])

            # ---- proj per q-range: wp stationary, out.T[of, q] --------------
            ofull = {}

            def proj_load(ri):
                q0, qw = QRANGES[ri]

                def u():
                    # ONE strided DMA fetches all 8 gathered shards:
                    # src [128, 8, qw] view of ag_out[ri], so the gpsimd
                    # queue never HOL-blocks an AllGather trigger behind
                    # 8 serialized load triggers.
                    t = ofull_pool.tile([P, 8 * qw], BF16, name="ofull")
                    nc.gpsimd.dma_start(
                        t[:].rearrange("p (c q) -> p c q", q=qw),
                        ag_out[ri].ap().rearrange("(c p) q -> p c q", p=P))
                    ofull[ri] = t
                return [u]

            def proj_units(ri):
                q0, qw = QRANGES[ri]
                units = []
                for h in range(2):
                    state = {}

                    def mk(h=h, half=0, state=state):
                        def u():
                            if half == 0:
                                state["ps"] = aux_pool.tile(
                                    [P, qw], F32, name="aux")
                            src = ofull[ri]
                            for cg in range(4 * half, 4 * half + 4):
                                nc.tensor.matmul(
                                    state["ps"][:], wp_sl(cg, h),
                                    src[:, cg * qw:(cg + 1) * qw],
                                    start=(cg == 0), stop=(cg == 7))
                            if half == 1:
                                pr = pr_pool.tile([P, qw], F32, name="pr")
                                nc.vector.tensor_scalar_add(
                                    pr[:], state["ps"][:], bp_sb[:, h:h + 1])
                                nc.gpsimd.dma_start(
                                    out_ext[h * P:(h + 1) * P, q0:q0 + qw],
                                    pr[:])
                        return u
                    units.append(mk(h, 0, state))
                    units.append(mk(h, 1, state))
                return units

            # ---- emission ---------------------------------------------------
            # Producers before consumers (Tile deps follow emission order).
            for u in qk_units(2, 0):
                u()
            for u in qk_units(0, 0):
                u()
            with tc.high_priority(offset=-1_000_000):
                fill = []
                fill += qk_units(2, 1)          # kt p0 n1 (attn(0,0) c4-7)
                fill += v_units(0) + v_units(1)
                fill += qk_units(2, 2)
                fill += v_units(2) + v_units(3)
                fill += qk_units(2, 3)
                fill += v_units(4) + v_units(5)
                fill += qk_units(0, 1)          # qt p0 n1 (range (0,1))
                fill += v_units(6) + v_units(7)
                fill += qk_units(3, 0)          # kt p1 (range (1,0))
                fill += v_units(8) + v_units(9)
                fill += qk_units(3, 1)
                fill += v_units(10) + v_units(11)
                fill += qk_units(3, 2)
                fill += v_units(12) + v_units(13)
                fill += qk_units(3, 3)
                fill += v_units(14) + v_units(15)
                fill += qk_units(1, 0)          # qt p1 n0
                fill += qk_units(0, 2)          # qt p0 n2
                fill += qk_units(1, 1)
                fill += qk_units(0, 3)
                fill += qk_units(1, 2)
                fill += qk_units(1, 3)
                for u in fill:
                    u()
            attn_range(0, 0)
            attn_range(0, 1)
            attn_range(1, 0)
            ag_range(0)
            with tc.high_priority(offset=-1_000_000):
                for u in proj_load(0):
                    u()
            attn_range(1, 1)
            ag_range(1)
            with tc.high_priority(offset=-1_000_000):
                for u in proj_load(1) + proj_units(0) + proj_units(1):
                    u()
            attn_range(0, 2)
            attn_range(1, 2)
            ag_range(2)
            with tc.high_priority(offset=-1_000_000):
                for u in proj_load(2) + proj_units(2):
                    u()
            attn_range(0, 5)
            attn_range(1, 3)
            ag_range(3)
            for u in proj_load(3):
                u()
            with tc.high_priority(offset=-1_000_000):
                for u in proj_units(3):
                    u()
            attn_range(1, 4)
            ag_range(4)
            for u in proj_load(4):
                u()
            with tc.high_priority(offset=-1_000_000):
                for u in proj_units(4):
                    u()

    nc.compile()
    return nc


_NC_CACHE = None


def _get_nc():
    global _NC_CACHE
    if _NC_CACHE is None:
        _NC_CACHE = build_nc()
    return _NC_CACHE


def _bf16(a):
    return np.ascontiguousarray(a.astype(ml_dtypes.bfloat16))


def kernel(x, w_qkv, b_qkv, w_proj, b_proj):
    global LAST_RESULTS
    x = np.asarray(x, dtype=np.float32)
    w_qkv = np.asarray(w_qkv, dtype=np.float32)
    b_qkv = np.asarray(b_qkv, dtype=np.float32)
    w_proj = np.asarray(w_proj, dtype=np.float32)
    b_proj = np.asarray(b_proj, dtype=np.float32)

    nc = _get_nc()

    in_maps = []
    for c in CORE_IDS:
        b, g = c // 4, c % 4
        cs = slice(g * OF, (g + 1) * OF)   # feature cols of this head group
        wq = w_qkv[:, 0 * D:1 * D][:, cs] * SCALE
        wk = w_qkv[:, 1 * D:2 * D][:, cs]
        wv = w_qkv[:, 2 * D:3 * D][:, cs]
        bq = b_qkv[0 * D:1 * D][cs] * SCALE
        bk = b_qkv[1 * D:2 * D][cs]
        bqk = np.concatenate([bq, bk]).reshape(4, P).T.copy()  # [128, 4]
        wqk = np.concatenate([wq, wk], axis=1)                 # [1024, 512]
        xt = x[b].T                                            # [1024, 2048]
        wp = w_proj[:, cs]                                     # [1024, 256]

        # packed layouts: k-chunk-major along the free dim
        xtp = np.empty((4 * P, DC * NT), np.float32)
        for n in range(4):
            for k in range(DC):
                xtp[n * P:(n + 1) * P, k * NT:(k + 1) * NT] = \
                    xt[k * P:(k + 1) * P, n * NT:(n + 1) * NT]
        wqkp = np.empty((4 * P, DC * P), np.float32)
        for m in range(4):
            for k in range(DC):
                wqkp[m * P:(m + 1) * P, k * P:(k + 1) * P] = \
                    wqk[k * P:(k + 1) * P, m * P:(m + 1) * P]
        wvp = np.empty((P, DC * OF), np.float32)
        wpp = np.empty((P, DC * OF), np.float32)
        for k in range(DC):
            wvp[:, k * OF:(k + 1) * OF] = wv[k * P:(k + 1) * P, :]
            wpp[:, k * OF:(k + 1) * OF] = wp[k * P:(k + 1) * P, :]

        in_maps.append({
            "xtp": _bf16(xtp),
            "wqkp": _bf16(wqkp),
            "wvp": _bf16(wvp),
            "wpp": _bf16(wpp),
            "bqk": np.ascontiguousarray(bqk, dtype=np.float32),
            "bv": np.ascontiguousarray(
                b_qkv[2 * D + g * OF:2 * D + (g + 1) * OF].reshape(1, OF)),
            "bp": np.ascontiguousarray(
                b_proj[cs].reshape(2, P).T, dtype=np.float32),
        })

    trace = bool(os.environ.get("KERNEL_TRACE"))
    if trace:
        _install_ntff_shim()
    LAST_RESULTS = run_bass_kernel_spmd(
        nc, in_maps, CORE_IDS, trace=trace)

    out = np.empty((B, N, D), dtype=np.float32)
    for c in CORE_IDS:
        b, g = c // 4, c % 4
        out[b, :, g * OF:(g + 1) * OF] = LAST_RESULTS.results[c]["out"].T
    return out
